# revision 16
# baseline (speedup 1.0000x reference)
"""BertAttention (abs-pos-emb variant) Trainium2 Bass kernel, 8-way batch-parallel.

Math (per batch item b, per head h):
    q = hidden @ Wq.T + bq ; k = ctx @ Wk.T + bk ; v = ctx @ Wv.T + bv
    scores = (q.k + (q+posq).posk)/8 + maskbias
           = (q/8).(k + posk) + (posq/8).posk + q.(bk)/8... (bk term is constant
             along k -> softmax-invariant -> dropped on device)
    out = softmax(scores) @ v

Device strategy (one core per batch item):
  - All matmuls in float32r (full-rate fp32, ~1.5e-4 rel err).
  - Q^T/K^T projections produce [h_out, s] tiles directly (lhsT = W.T chunks,
    rhs = hidden^T/ctx^T chunks).  V is produced in natural [sk, h_out] layout
    (lhsT = ctx^T chunks, rhs = Wv.T).
  - Per head, augmented 128-row contraction computes S^T[k, q] in one matmul
    per k-block: rows = [K^T[h] + posk^T ; posk^T] vs [Q^T[h]/8 + bq/8 ; posq^T/8]
    (halves swapped for odd heads so PSUM de-interleave never crosses
    partition bases).
  - E^T = exp(S^T) via ScalarE; no row-max subtraction (|scores| <= ~15 << 88,
    so exp cannot overflow).  The 0/1 attention mask is folded into Vaug
    instead of the scores: Vaug = [V * mask ; mask], so both the AV product
    and the softmax denominator come out masked.
  - O^T[65, q] = Vaug^T @ E^T accumulated over k-blocks: rows 0-63 are the
    unnormalized head output, row 64 the masked softmax denominator.
  - Host divides by the denominator, adds bv, and re-lays-out to [B, SQ, H].
  - Projections run ko-outer (all 6 output tiles accumulate in parallel as
    each 128-row contraction chunk's DMA lands); heads 0-3 are scored during
    the DMA-starved projection phase; AV is software-pipelined 2 heads behind
    scoring.  Modeled per-core exec: ~68 us (PE-bound at ~47 us busy).
"""

import numpy as np

import concourse.bass as bass
import concourse.mybir as mybir
import concourse.tile as tile
from concourse import bacc
from concourse.bass_utils import run_bass_kernel_spmd

B, SQ, SK, H, NH, DH = 8, 512, 512, 768, 12, 64
P = 128
KO = H // P          # 6 contraction chunks of 128
NKB = SK // P        # 4 key blocks
N_CORES = 8
VN = 384             # V projection free-dim half (768 = 2 x 384, both >=256)
F32 = mybir.dt.float32
F32R = mybir.dt.float32r

TRACE = False           # set by test harness for profiled runs
_last_results = None    # BassKernelResults of the most recent run
_nc = None              # cached compiled Bass module


def _build(cfg=None):
    cfg = cfg or {}
    dma_order = cfg.get("dma_order", "priority")   # "priority" | "zipper"
    ko_outer = cfg.get("ko_outer", True)           # ko-outer projection loops
    early_heads = cfg.get("early_heads", 4)        # heads scored during proj
    lookahead = cfg.get("lookahead", 2)            # AV pipeline distance
    e_bufs = cfg.get("e_bufs", 4)
    exp_pair = cfg.get("exp_pair", 1)   # kb tiles fused per exp (1 or 2)
    q_add_act = cfg.get("q_add_act", True)   # Q bias add on ScalarE
    pps_bufs = cfg.get("pps_bufs", 7)
    sps_bufs = cfg.get("sps_bufs", 7)
    ops_bufs = cfg.get("ops_bufs", 1)
    ps_tag = cfg.get("ps_tag", "shared")  # one rotating 1-bank PSUM tag            # "split" | "shared"

    nc = bacc.Bacc("TRN2", target_bir_lowering=False, debug=False)

    def din(name, shape, dt=F32R):
        return nc.dram_tensor(name, shape, dt, kind="ExternalInput").ap()

    hsT = din("hsT", [H, SQ])          # hidden[b].T
    ctxT = din("ctxT", [H, SK])        # context[b].T
    wq = din("wq", [H, H])             # Wq.T / 8
    wk = din("wk", [H, H])             # Wk.T
    wv = din("wv", [H, H])             # Wv.T
    posq = din("posq", [P, SQ])        # (posq/8).T stacked twice vertically
    posk = din("posk", [P, SK])        # posk.T stacked twice vertically
    maskb = din("maskb", [P, NKB], F32)  # 0/1 mask, [ki, ko]
    bq8 = din("bq8", [P, KO], F32)       # bq/8, [p, mo]
    out = nc.dram_tensor("out", [DH + 1, NH, SQ], F32, kind="ExternalOutput").ap()

    hsT_r = hsT.rearrange("(ko ki) s -> ki ko s", ki=P)
    ctxT_r = ctxT.rearrange("(ko ki) s -> ki ko s", ki=P)
    wq_r = wq.rearrange("(ko ki) m -> ki ko m", ki=P)
    wk_r = wk.rearrange("(ko ki) m -> ki ko m", ki=P)
    wv_r = wv.rearrange("(ko ki) m -> ki ko m", ki=P)

    Add = mybir.AluOpType.add
    Exp = mybir.ActivationFunctionType.Exp

    with tile.TileContext(nc) as tc:
        with tc.tile_pool(name="pin", bufs=1) as pin, \
             tc.tile_pool(name="pqk", bufs=1) as pqk, \
             tc.tile_pool(name="pe", bufs=2) as pe_pool, \
             tc.tile_pool(name="pout", bufs=1) as pout, \
             tc.tile_pool(name="ps", bufs=1, space="PSUM") as ps:

            hsT_sb = pin.tile([P, KO, SQ], F32R, name="hsT_sb", tag="hsT")
            ctxT_sb = pin.tile([P, KO, SK], F32R, name="ctxT_sb", tag="ctxT")
            wq_sb = pin.tile([P, KO, H], F32R, name="wq_sb", tag="wq")
            wk_sb = pin.tile([P, KO, H], F32R, name="wk_sb", tag="wk")
            wv_sb = pin.tile([P, KO, H], F32R, name="wv_sb", tag="wv")
            maskb_sb = pin.tile([P, NKB], F32, name="maskb_sb", tag="maskb")
            bq8_sb = pin.tile([P, KO], F32, name="bq8_sb", tag="bq8")
            posq_sb = pin.tile([P, SQ], F32R, name="posq_sb", tag="posq")
            posk_sb = pin.tile([P, SK], F32R, name="posk_sb", tag="posk")

            if cfg.get("split_first", False):
                nc.sync.dma_start(hsT_sb[:, 0, :], hsT_r[:, 0, :])
                nc.sync.dma_start(wq_sb[:, 0, 0:256], wq_r[:, 0, 0:256])
                nc.sync.dma_start(wq_sb[:, 0, 256:512], wq_r[:, 0, 256:512])
                nc.sync.dma_start(wq_sb[:, 0, 512:768], wq_r[:, 0, 512:768])
            else:
                nc.sync.dma_start(wq_sb[:, 0, :], wq_r[:, 0, :])
                nc.sync.dma_start(hsT_sb[:, 0, :], hsT_r[:, 0, :])
            nc.sync.dma_start(bq8_sb[:], bq8)
            nc.sync.dma_start(posq_sb[:], posq)
            nc.sync.dma_start(maskb_sb[:], maskb)
            nc.sync.dma_start(posk_sb[:], posk)
            if dma_order == "priority":
                for ko in range(1, KO):
                    nc.sync.dma_start(wq_sb[:, ko, :], wq_r[:, ko, :])
                    nc.sync.dma_start(hsT_sb[:, ko, :], hsT_r[:, ko, :])
                for ko in range(KO):
                    nc.sync.dma_start(ctxT_sb[:, ko, :], ctxT_r[:, ko, :])
                    nc.sync.dma_start(wk_sb[:, ko, :], wk_r[:, ko, :])
            else:
                nc.sync.dma_start(ctxT_sb[:, 0, :], ctxT_r[:, 0, :])
                nc.sync.dma_start(wk_sb[:, 0, :], wk_r[:, 0, :])
                for ko in range(1, KO):
                    nc.sync.dma_start(wq_sb[:, ko, :], wq_r[:, ko, :])
                    nc.sync.dma_start(hsT_sb[:, ko, :], hsT_r[:, ko, :])
                    nc.sync.dma_start(ctxT_sb[:, ko, :], ctxT_r[:, ko, :])
                    nc.sync.dma_start(wk_sb[:, ko, :], wk_r[:, ko, :])
            for ko in range(KO):
                nc.sync.dma_start(wv_sb[:, ko, :], wv_r[:, ko, :])

            Qa = [pqk.tile([P, SQ], F32R, name=f"qa{h}", tag=f"qa{h}")
                  for h in range(NH)]
            Ka = [pqk.tile([P, SK], F32R, name=f"ka{h}", tag=f"ka{h}")
                  for h in range(NH)]
            vaug = pqk.tile([P, NKB, NH, DH + 1], F32R, name="vaug", tag="vaug")

            def q_copies(mo, q_ps):
                for half in range(2):
                    h = 2 * mo + half
                    sl = slice(half * DH, (half + 1) * DH)       # Q rows
                    osl = slice(DH - half * DH, 2 * DH - half * DH)  # posq rows
                    if q_add_act:
                        nc.scalar.add(Qa[h][sl, :], q_ps[sl, :], bq8_sb[sl, mo:mo + 1])
                    else:
                        nc.vector.tensor_scalar_add(Qa[h][sl, :], q_ps[sl, :], bq8_sb[sl, mo:mo + 1])
                    nc.vector.tensor_copy(Qa[h][osl, :], posq_sb[osl, :])

            def q_proj(mo):
                q_ps = ps.tile([P, SQ], F32, name="q_ps", tag=("ps" if ps_tag == "shared" else "pps"), bufs=pps_bufs)
                for ko in range(KO):
                    nc.tensor.matmul(q_ps[:], wq_sb[:, ko, mo * P:(mo + 1) * P],
                                     hsT_sb[:, ko, :],
                                     start=(ko == 0), stop=(ko == KO - 1))
                q_copies(mo, q_ps)

            def k_copies(mo, k_ps):
                for half in range(2):
                    h = 2 * mo + half
                    sl = slice(half * DH, (half + 1) * DH)
                    osl = slice(DH - half * DH, 2 * DH - half * DH)
                    nc.vector.tensor_tensor(Ka[h][sl, :], k_ps[sl, :],
                                            posk_sb[sl, :], Add)
                    nc.vector.tensor_copy(Ka[h][osl, :], posk_sb[osl, :])

            def k_proj(mo):
                k_ps = ps.tile([P, SK], F32, name="k_ps", tag=("ps" if ps_tag == "shared" else "pps"), bufs=pps_bufs)
                for ko in range(KO):
                    nc.tensor.matmul(k_ps[:], wk_sb[:, ko, mo * P:(mo + 1) * P],
                                     ctxT_sb[:, ko, :],
                                     start=(ko == 0), stop=(ko == KO - 1))
                k_copies(mo, k_ps)

            def qk_proj_ko_outer(w_sb, x_sb, copies, n_free):
                W = min(pps_bufs, KO)
                for w0 in range(0, KO, W):
                    mos = list(range(w0, min(w0 + W, KO)))
                    tiles = {mo: ps.tile([P, n_free], F32, name=f"p{mo}",
                                         tag=("ps" if ps_tag == "shared" else "pps"),
                                         bufs=pps_bufs) for mo in mos}
                    for ko in range(KO):
                        for mo in mos:
                            nc.tensor.matmul(tiles[mo][:],
                                             w_sb[:, ko, mo * P:(mo + 1) * P],
                                             x_sb[:, ko, :],
                                             start=(ko == 0), stop=(ko == KO - 1))
                    for mo in mos:
                        copies(mo, tiles[mo])

            def v_proj():
                for so in range(NKB):
                    for half in range(2):
                        v_ps = ps.tile([P, VN], F32, name="v_ps", tag=("ps" if ps_tag == "shared" else "pps"), bufs=pps_bufs)
                        for ko in range(KO):
                            nc.tensor.matmul(
                                v_ps[:], ctxT_sb[:, ko, so * P:(so + 1) * P],
                                wv_sb[:, ko, half * VN:(half + 1) * VN],
                                start=(ko == 0), stop=(ko == KO - 1))
                        nc.vector.tensor_scalar_mul(
                            vaug[:, so,
                                 half * (VN // DH):(half + 1) * (VN // DH), 0:DH],
                            v_ps[:].rearrange("p (h d) -> p h d", d=DH),
                            maskb_sb[:, so:so + 1])
                for so in range(NKB):
                    nc.vector.tensor_copy(
                        vaug[:, so, :, DH],
                        maskb_sb[:, so:so + 1].to_broadcast([P, NH]))

            def s_exp(h):
                es = []
                for grp in range(NKB // exp_pair):
                    s_ps = ps.tile([P, exp_pair, SQ], F32, name="s_ps",
                                   tag=("ps" if ps_tag == "shared" else "sps"),
                                   bufs=sps_bufs)
                    for half in range(exp_pair):
                        kb = exp_pair * grp + half
                        nc.tensor.matmul(s_ps[:, half, :],
                                         Ka[h][:, kb * P:(kb + 1) * P],
                                         Qa[h][:], start=True, stop=True)
                    e = pe_pool.tile([P, exp_pair, SQ], F32R, name=f"e{grp}",
                                     tag=f"e{grp}", bufs=e_bufs)
                    nc.scalar.activation(e[:], s_ps[:], Exp, scale=1.0)
                    es.append(e)
                return es

            def av(h, es):
                o_ps = ps.tile([DH + 1, SQ], F32, name="o_ps",
                               tag=("ps" if cfg.get("ops_shared") else "ops"),
                               bufs=ops_bufs)
                for kb in range(NKB):
                    nc.tensor.matmul(o_ps[:], vaug[:, kb, h, :],
                                     es[kb // exp_pair][:, kb % exp_pair, :],
                                     start=(kb == 0), stop=(kb == NKB - 1))
                o_sb = pout.tile([DH + 1, SQ], F32, name="o_sb", tag="o_sb",
                                 bufs=2)
                nc.vector.tensor_copy(o_sb[:], o_ps[:])
                nc.sync.dma_start(out[:, h, :], o_sb[:])

            E = {}
            if ko_outer:
                qk_proj_ko_outer(wq_sb, hsT_sb, q_copies, SQ)
                qk_proj_ko_outer(wk_sb, ctxT_sb, k_copies, SK)
                for h in range(early_heads):
                    E[h] = s_exp(h)
            else:
                for mo in range(KO):
                    q_proj(mo)
                    k_proj(mo)
                    if 2 * mo < early_heads:
                        E[2 * mo] = s_exp(2 * mo)
                    if 2 * mo + 1 < early_heads:
                        E[2 * mo + 1] = s_exp(2 * mo + 1)
            v_proj()
            next_s = early_heads
            next_av = 0
            while next_av < NH:
                if next_s < NH and next_s - next_av < lookahead:
                    E[next_s] = s_exp(next_s)
                    next_s += 1
                else:
                    av(next_av, E.pop(next_av))
                    next_av += 1

    nc.finalize()
    return nc


def _prep_inputs(hidden_states, context, attention_mask, Wq, bq, Wk, Wv,
                 abs_pos_emb):
    f32 = np.float32
    pos = np.asarray(abs_pos_emb, f32)[:SQ]          # [512, 64]
    posqT = np.ascontiguousarray((pos / 8.0).T)       # [64, 512]
    poskT = np.ascontiguousarray(pos.T)
    posq_dup = np.concatenate([posqT, posqT], axis=0)  # [128, 512]
    posk_dup = np.concatenate([poskT, poskT], axis=0)
    wq8 = np.ascontiguousarray(np.asarray(Wq, f32).T / 8.0)
    wkT = np.ascontiguousarray(np.asarray(Wk, f32).T)
    wvT = np.ascontiguousarray(np.asarray(Wv, f32).T)
    bq8_r = np.ascontiguousarray((np.asarray(bq, f32) / 8.0).reshape(KO, P).T)
    hs = np.asarray(hidden_states, f32)
    ctx = np.asarray(context, f32)
    am = np.asarray(attention_mask)

    in_maps = []
    for c in range(N_CORES):
        mb = (am[c] != 0).astype(f32)
        in_maps.append({
            "hsT": np.ascontiguousarray(hs[c].T),
            "ctxT": np.ascontiguousarray(ctx[c].T),
            "wq": wq8, "wk": wkT, "wv": wvT,
            "posq": posq_dup, "posk": posk_dup,
            "maskb": np.ascontiguousarray(mb.reshape(NKB, P).T),
            "bq8": bq8_r,
        })
    return in_maps


def kernel(hidden_states, context, attention_mask, Wq, bq, Wk, bk, Wv, bv,
           abs_pos_emb):
    global _nc, _last_results
    if _nc is None:
        _nc = _build()
    in_maps = _prep_inputs(hidden_states, context, attention_mask,
                           Wq, bq, Wk, Wv, abs_pos_emb)
    res = run_bass_kernel_spmd(_nc, in_maps, core_ids=list(range(N_CORES)),
                               trace=TRACE)
    _last_results = res

    bv_f = np.asarray(bv, np.float32)
    outs = np.empty((B, SQ, H), np.float32)
    for c in range(N_CORES):
        buf = np.asarray(res.results[c]["out"])       # [65, NH, SQ]
        o = buf[:DH] / buf[DH:DH + 1]                 # [64, NH, SQ]
        outs[c] = o.transpose(2, 1, 0).reshape(SQ, H) + bv_f[None, :]
    return outs
